# revision 3
# baseline (speedup 1.0000x reference)
"""Chamfer loss (nn_ChamferLoss) on 8 Trainium2 NeuronCores.

Strategy
--------
loss = 2 * mean_b( sum_n min_m ||pos1[b,n] - pos2[b,m]||^2 ), pos1 = pc2^T,
pos2 = pc1_warped^T, B=4, N=M=8192, C=3.

Sharding: core c = 2*b + h handles batch b, query half h (4096 queries)
against batch b's refs (data parallel over B plus a query split — 8 cores).

Device kernel (identical SPMD program on all cores; all data-dependence
lives in the input *contents*):
  * Host sorts queries and refs of each batch along coordinate 0. Each
    core's 4096 sorted queries only need refs near them in sorted order,
    so the host ships a contiguous ref "slab" (3968 + W sorted refs,
    edge-replicated at array bounds) pre-shifted per core. Query block
    j (128 queries) searches the W-wide window starting at slab offset
    128*j — a static offset, identical on every core.
  * Distances via one K=4 augmented matmul: lhsT rows [qx,qy,qz,1],
    rhs rows [2rx,2ry,2rz,-||r||^2] ->  psum[n,m] = 2 q.r - r2
    = q2[n] - d[n,m].  reduce_max over the window gives
    M[n] = q2[n] - min_m d[n,m]; host recovers nn[n] = max(q2[n]-M[n],0).
  * Exactness: for each query the host checks the certificate
    nn <= (distance along the sort axis to the nearest ref *outside*
    the searched window)^2. Certified queries provably found the global
    min. The few uncertified ones (~4% at W=1024) are recomputed exactly
    on the host. The result is exact brute force, not approximate.
"""

import os

import numpy as np

_B, _C, _N = 4, 3, 8192
_NCORES = 8
_QB = 128                       # queries per block (psum partitions)
_NQ_CORE = _N // 2              # queries per core
_NQB = _NQ_CORE // _QB          # query blocks per core (32)
_W = int(os.environ.get("CHAMFER_W", "1024"))      # ref window per block
_SLAB = _NQ_CORE - _QB + _W     # ref slab length per core
_MM = 512                       # moving-operand free-dim max (fp32)

_prog_cache = {}
LAST_RESULT = None              # BassKernelResults of the last run (for tests)


def _get_program(w):
    """Build (once) the SPMD bass program. Fully data-independent."""
    if w in _prog_cache:
        return _prog_cache[w]

    import concourse.bacc as bacc
    import concourse.tile as tile
    from concourse import mybir

    slab = _NQ_CORE - _QB + w
    nc = bacc.Bacc("TRN2", target_bir_lowering=False, debug=False)
    f32 = mybir.dt.float32
    lhsT_d = nc.dram_tensor("lhsT", [4, _NQ_CORE], f32, kind="ExternalInput")
    rhs_d = nc.dram_tensor("rhs", [4, slab], f32, kind="ExternalInput")
    mins_d = nc.dram_tensor("mins", [_QB, _NQB], f32, kind="ExternalOutput")

    with tile.TileContext(nc) as tc:
        with (
            tc.tile_pool(name="consts", bufs=1) as consts,
            tc.tile_pool(name="psum", bufs=3, space="PSUM") as psum_pool,
        ):
            lhsT_sb = consts.tile([4, _NQ_CORE], f32)
            rhs_sb = consts.tile([4, slab], f32)
            out_sb = consts.tile([_QB, _NQB], f32)

            # chunked loads so several DMA queues run in parallel
            ch = 1024
            for s in range(0, _NQ_CORE, ch):
                e = min(s + ch, _NQ_CORE)
                nc.sync.dma_start(out=lhsT_sb[:, s:e], in_=lhsT_d[:, s:e])
            for s in range(0, slab, ch):
                e = min(s + ch, slab)
                nc.sync.dma_start(out=rhs_sb[:, s:e], in_=rhs_d[:, s:e])

            for j in range(_NQB):
                ps = psum_pool.tile([_QB, w], f32)
                for t in range(0, w, _MM):
                    te = min(t + _MM, w)
                    nc.tensor.matmul(
                        ps[:, t:te],
                        lhsT_sb[:, j * _QB : (j + 1) * _QB],
                        rhs_sb[:, j * _QB + t : j * _QB + te],
                        start=True,
                        stop=True,
                    )
                nc.vector.tensor_reduce(
                    out_sb[:, j : j + 1],
                    ps[:],
                    axis=mybir.AxisListType.X,
                    op=mybir.AluOpType.max,
                )

            nc.sync.dma_start(out=mins_d[:], in_=out_sb[:])

    nc.compile()
    _prog_cache[w] = nc
    return nc


def _install_axon_ntff_hook():
    """Dev-only (CHAMFER_TRACE=1): bridge the missing antenv.axon_hooks
    module so run_bass_kernel_spmd's axon trace path can capture NTFFs."""
    import sys
    import types

    if "antenv.axon_hooks" in sys.modules:
        return
    try:
        from trn_agent_boot.trn_boot import _ntff_profile_via_ctypes

        hook = _ntff_profile_via_ctypes("/opt/axon/libaxon_pjrt.so")
    except Exception:
        hook = None
    mod = types.ModuleType("antenv.axon_hooks")
    mod.get_axon_ntff_profile_hook = lambda: hook
    mod.set_axon_ntff_profile_hook = lambda h: None
    sys.modules["antenv.axon_hooks"] = mod


def _exact_nn(q, r):
    """Exact fallback, mirrors the reference's fp32 arithmetic.
    q: [3, nq] queries, r: [3, N] refs -> [nq] min sq dists (fp32)."""
    q = np.asarray(q, np.float32)
    r = np.asarray(r, np.float32)
    q2 = (q * q).sum(0)
    r2 = (r * r).sum(0)
    out = np.empty(q.shape[1], np.float32)
    for s in range(0, q.shape[1], 1024):
        e = min(s + 1024, q.shape[1])
        cross = q[:, s:e].T @ r
        d = q2[s:e, None] + r2[None, :] - 2.0 * cross
        np.maximum(d, 0.0, out=d)
        out[s:e] = d.min(1)
    return out


def kernel(pc2, pc1_warped):
    from concourse.bass_utils import run_bass_kernel_spmd

    global LAST_RESULT
    pc2 = np.ascontiguousarray(np.asarray(pc2), dtype=np.float32)
    pc1w = np.ascontiguousarray(np.asarray(pc1_warped), dtype=np.float32)
    B, C, N = pc2.shape
    assert (B, C, N) == (_B, _C, _N), f"unexpected shape {pc2.shape}"
    w = _W
    half_w = w // 2

    in_maps = []
    meta = []
    ones = np.ones((1, _NQ_CORE), np.float32)
    for b in range(B):
        qidx = np.argsort(pc2[b, 0], kind="stable")
        ridx = np.argsort(pc1w[b, 0], kind="stable")
        qs = pc2[b][:, qidx]                 # [3, N] sorted queries
        rs = pc1w[b][:, ridx]                # [3, N] sorted refs
        q2s = (qs * qs).sum(0)               # [N]
        r2s = (rs * rs).sum(0)
        for h in range(2):
            lq = qs[:, h * _NQ_CORE : (h + 1) * _NQ_CORE]
            lhsT = np.concatenate([lq, ones], 0).astype(np.float32)
            slab_start = _NQ_CORE * h + _QB // 2 - half_w
            sidx = np.clip(np.arange(slab_start, slab_start + _SLAB), 0, N - 1)
            rhs = np.concatenate(
                [2.0 * rs[:, sidx], -(r2s[sidx])[None, :]], 0
            ).astype(np.float32)
            in_maps.append({"lhsT": lhsT, "rhs": rhs})
            meta.append((b, h, slab_start))
        # stash per-batch arrays for the certify/unshard pass
        meta[-1] = meta[-1] + (qs, rs, q2s)
        meta[-2] = meta[-2] + (qs, rs, q2s)

    nc = _get_program(w)
    trace = os.environ.get("CHAMFER_TRACE") == "1"
    kwargs = {}
    if trace:
        _install_axon_ntff_hook()
        kwargs = dict(trace=True, trace_cores=[0])
    res = run_bass_kernel_spmd(nc, in_maps, list(range(_NCORES)), **kwargs)
    LAST_RESULT = res

    total = np.float64(0.0)
    arange_qb = np.arange(_QB)
    for c in range(_NCORES):
        b, h, slab_start, qs, rs, q2s = meta[c]
        zq = qs[0]
        zr = rs[0]
        M = np.asarray(res.results[c]["mins"], np.float32)     # [128, 32]
        # element [p, j] is query (sorted) rank h*4096 + j*128 + p
        Mq = M.T.reshape(-1)                                   # [4096] rank order
        ranks = h * _NQ_CORE + np.arange(_NQ_CORE)
        nn = np.maximum(q2s[ranks].astype(np.float64) - Mq.astype(np.float64), 0.0)

        # certificates, per block
        uncert = np.zeros(_NQ_CORE, bool)
        for j in range(_NQB):
            rk = h * _NQ_CORE + j * _QB + arange_qb
            glo = max(slab_start + j * _QB, 0)
            ghi = min(slab_start + j * _QB + w - 1, N - 1)
            lo_m = (zq[rk] - zr[glo - 1]) if glo > 0 else np.full(_QB, np.inf)
            hi_m = (zr[ghi + 1] - zq[rk]) if ghi < N - 1 else np.full(_QB, np.inf)
            guard = np.minimum(lo_m, hi_m)
            bad = ~((guard >= 0) & (nn[j * _QB + arange_qb] <= guard * guard))
            uncert[j * _QB + arange_qb] = bad

        nu = int(uncert.sum())
        if nu:
            # exact host recompute against the batch's full ref set
            qu = qs[:, h * _NQ_CORE + np.nonzero(uncert)[0]]
            nn[uncert] = _exact_nn(qu, rs).astype(np.float64)
        total += nn.sum()

    loss = (2.0 / _B) * total
    return np.float32(loss)


# revision 5
# speedup vs baseline: 1.2711x; 1.2711x over previous
"""Chamfer loss (nn_ChamferLoss) on 8 Trainium2 NeuronCores.

Strategy
--------
loss = 2 * mean_b( sum_n min_m ||pos1[b,n] - pos2[b,m]||^2 ), pos1 = pc2^T,
pos2 = pc1_warped^T, B=4, N=M=8192, C=3.

Sharding: core c = 2*b + h handles batch b, query half h (4096 queries)
against batch b's refs (data parallel over B plus a query split — 8 cores).

Device kernel (identical SPMD program on all cores; all data-dependence
lives in the input *contents*):
  * Host sorts queries and refs of each batch along coordinate 0. Each
    core's 4096 sorted queries only need refs near them in sorted order,
    so the host ships a contiguous ref "slab" (3968 + W sorted refs,
    edge-replicated at array bounds) pre-shifted per core. Query block
    j (128 queries) searches the W-wide window starting at slab offset
    128*j — a static offset, identical on every core.
  * Distances via one K=4 augmented matmul: lhsT rows [qx,qy,qz,1],
    rhs rows [2rx,2ry,2rz,-||r||^2] ->  psum[n,m] = 2 q.r - r2
    = q2[n] - d[n,m].  reduce_max over the window gives
    M[n] = q2[n] - min_m d[n,m]; host recovers nn[n] = max(q2[n]-M[n],0).
  * Exactness: for each query the host checks the certificate
    nn <= (distance along the sort axis to the nearest ref *outside*
    the searched window)^2. Certified queries provably found the global
    min. The few uncertified ones (~4% at W=1024) are recomputed exactly
    on the host. The result is exact brute force, not approximate.
"""

import os

import numpy as np

_B, _C, _N = 4, 3, 8192
_NCORES = 8
_QB = 128                       # queries per block (psum partitions)
_NQ_CORE = _N // 2              # queries per core
_NQB = _NQ_CORE // _QB          # query blocks per core (32)
_W = int(os.environ.get("CHAMFER_W", "1024"))      # ref window per block
_SLAB = _NQ_CORE - _QB + _W     # ref slab length per core
_MM = 512                       # moving-operand free-dim max (fp32)

_prog_cache = {}
LAST_RESULT = None              # BassKernelResults of the last run (for tests)


def _get_program(w):
    """Build (once) the SPMD bass program. Fully data-independent.

    fp16 hi/lo split: the PE runs fp32 matmuls ~5x slower than 16-bit, so
    the K=4 augmented operands are shipped as fp16 (hi, lo) pairs and each
    512-wide psum chunk accumulates three fp16 matmuls:
        hi.hi + hi.lo + lo.hi   (the lo.lo term is ~2^-22 — dropped)
    which reproduces the fp32 product to ~1e-5 absolute.
    """
    if w in _prog_cache:
        return _prog_cache[w]

    import concourse.bacc as bacc
    import concourse.tile as tile
    from concourse import mybir

    slab = _NQ_CORE - _QB + w
    nc = bacc.Bacc("TRN2", target_bir_lowering=False, debug=False)
    f32 = mybir.dt.float32
    f16 = mybir.dt.float16
    lh_d = nc.dram_tensor("lhsT_h", [4, _NQ_CORE], f16, kind="ExternalInput")
    ll_d = nc.dram_tensor("lhsT_l", [4, _NQ_CORE], f16, kind="ExternalInput")
    rh_d = nc.dram_tensor("rhs_h", [4, slab], f16, kind="ExternalInput")
    rl_d = nc.dram_tensor("rhs_l", [4, slab], f16, kind="ExternalInput")
    mins_d = nc.dram_tensor("mins", [_QB, _NQB], f32, kind="ExternalOutput")

    with tile.TileContext(nc) as tc:
        with (
            tc.tile_pool(name="consts", bufs=1) as consts,
            tc.tile_pool(name="psum", bufs=3, space="PSUM") as psum_pool,
        ):
            lh_sb = consts.tile([4, _NQ_CORE], f16)
            ll_sb = consts.tile([4, _NQ_CORE], f16)
            rh_sb = consts.tile([4, slab], f16)
            rl_sb = consts.tile([4, slab], f16)
            out_sb = consts.tile([_QB, _NQB], f32)

            # chunked loads so several DMA queues run in parallel
            ch = 2048
            for sb_t, d_t, n in (
                (lh_sb, lh_d, _NQ_CORE),
                (ll_sb, ll_d, _NQ_CORE),
                (rh_sb, rh_d, slab),
                (rl_sb, rl_d, slab),
            ):
                for s in range(0, n, ch):
                    e = min(s + ch, n)
                    nc.sync.dma_start(out=sb_t[:, s:e], in_=d_t[:, s:e])

            for j in range(_NQB):
                ps = psum_pool.tile([_QB, w], f32)
                qsl = slice(j * _QB, (j + 1) * _QB)
                for t in range(0, w, _MM):
                    te = min(t + _MM, w)
                    rsl = slice(j * _QB + t, j * _QB + te)
                    nc.tensor.matmul(
                        ps[:, t:te], lh_sb[:, qsl], rh_sb[:, rsl],
                        start=True, stop=False,
                    )
                    nc.tensor.matmul(
                        ps[:, t:te], lh_sb[:, qsl], rl_sb[:, rsl],
                        start=False, stop=False,
                    )
                    nc.tensor.matmul(
                        ps[:, t:te], ll_sb[:, qsl], rh_sb[:, rsl],
                        start=False, stop=True,
                    )
                nc.vector.tensor_reduce(
                    out_sb[:, j : j + 1],
                    ps[:],
                    axis=mybir.AxisListType.X,
                    op=mybir.AluOpType.max,
                )

            nc.sync.dma_start(out=mins_d[:], in_=out_sb[:])

    nc.compile()
    _prog_cache[w] = nc
    return nc


def _split16(a):
    """fp32 array -> (hi, lo) fp16 pair with hi + lo ~= a."""
    hi = a.astype(np.float16)
    lo = (a - hi.astype(np.float32)).astype(np.float16)
    return hi, lo


def _install_axon_ntff_hook():
    """Dev-only (CHAMFER_TRACE=1): bridge the missing antenv.axon_hooks
    module so run_bass_kernel_spmd's axon trace path can capture NTFFs."""
    import sys
    import types

    if "antenv.axon_hooks" in sys.modules:
        return
    try:
        from trn_agent_boot.trn_boot import _ntff_profile_via_ctypes

        hook = _ntff_profile_via_ctypes("/opt/axon/libaxon_pjrt.so")
    except Exception:
        hook = None
    mod = types.ModuleType("antenv.axon_hooks")
    mod.get_axon_ntff_profile_hook = lambda: hook
    mod.set_axon_ntff_profile_hook = lambda h: None
    sys.modules["antenv.axon_hooks"] = mod


def _exact_nn(q, r):
    """Exact fallback, mirrors the reference's fp32 arithmetic.
    q: [3, nq] queries, r: [3, N] refs -> [nq] min sq dists (fp32)."""
    q = np.asarray(q, np.float32)
    r = np.asarray(r, np.float32)
    q2 = (q * q).sum(0)
    r2 = (r * r).sum(0)
    out = np.empty(q.shape[1], np.float32)
    for s in range(0, q.shape[1], 1024):
        e = min(s + 1024, q.shape[1])
        cross = q[:, s:e].T @ r
        d = q2[s:e, None] + r2[None, :] - 2.0 * cross
        np.maximum(d, 0.0, out=d)
        out[s:e] = d.min(1)
    return out


def kernel(pc2, pc1_warped):
    from concourse.bass_utils import run_bass_kernel_spmd

    global LAST_RESULT
    pc2 = np.ascontiguousarray(np.asarray(pc2), dtype=np.float32)
    pc1w = np.ascontiguousarray(np.asarray(pc1_warped), dtype=np.float32)
    B, C, N = pc2.shape
    assert (B, C, N) == (_B, _C, _N), f"unexpected shape {pc2.shape}"
    w = _W
    half_w = w // 2

    in_maps = []
    meta = []
    ones = np.ones((1, _NQ_CORE), np.float32)
    zeros = np.zeros((1, _NQ_CORE), np.float16)
    for b in range(B):
        qidx = np.argsort(pc2[b, 0], kind="stable")
        ridx = np.argsort(pc1w[b, 0], kind="stable")
        qs = pc2[b][:, qidx]                 # [3, N] sorted queries
        rs = pc1w[b][:, ridx]                # [3, N] sorted refs
        q2s = (qs * qs).sum(0)               # [N]
        r2s = (rs * rs).sum(0)
        for h in range(2):
            lq = qs[:, h * _NQ_CORE : (h + 1) * _NQ_CORE]
            qh, ql = _split16(lq)
            # ones row rides in the hi part only; lo part gets zeros
            lhsT_h = np.concatenate([qh, ones.astype(np.float16)], 0)
            lhsT_l = np.concatenate([ql, zeros], 0)
            slab_start = _NQ_CORE * h + _QB // 2 - half_w
            sidx = np.clip(np.arange(slab_start, slab_start + _SLAB), 0, N - 1)
            Rh, Rl = _split16(2.0 * rs[:, sidx])
            r2h, r2l = _split16(-(r2s[sidx])[None, :])
            rhs_h = np.concatenate([Rh, r2h], 0)
            rhs_l = np.concatenate([Rl, r2l], 0)
            in_maps.append({"lhsT_h": lhsT_h, "lhsT_l": lhsT_l,
                            "rhs_h": rhs_h, "rhs_l": rhs_l})
            meta.append((b, h, slab_start))
        # stash per-batch arrays for the certify/unshard pass
        meta[-1] = meta[-1] + (qs, rs, q2s)
        meta[-2] = meta[-2] + (qs, rs, q2s)

    nc = _get_program(w)
    trace = os.environ.get("CHAMFER_TRACE") == "1"
    kwargs = {}
    if trace:
        _install_axon_ntff_hook()
        kwargs = dict(trace=True, trace_cores=[0])
    res = run_bass_kernel_spmd(nc, in_maps, list(range(_NCORES)), **kwargs)
    LAST_RESULT = res

    total = np.float64(0.0)
    arange_qb = np.arange(_QB)
    for c in range(_NCORES):
        b, h, slab_start, qs, rs, q2s = meta[c]
        zq = qs[0]
        zr = rs[0]
        M = np.asarray(res.results[c]["mins"], np.float32)     # [128, 32]
        # element [p, j] is query (sorted) rank h*4096 + j*128 + p
        Mq = M.T.reshape(-1)                                   # [4096] rank order
        ranks = h * _NQ_CORE + np.arange(_NQ_CORE)
        nn = np.maximum(q2s[ranks].astype(np.float64) - Mq.astype(np.float64), 0.0)

        # certificates, per block
        uncert = np.zeros(_NQ_CORE, bool)
        for j in range(_NQB):
            rk = h * _NQ_CORE + j * _QB + arange_qb
            glo = max(slab_start + j * _QB, 0)
            ghi = min(slab_start + j * _QB + w - 1, N - 1)
            lo_m = (zq[rk] - zr[glo - 1]) if glo > 0 else np.full(_QB, np.inf)
            hi_m = (zr[ghi + 1] - zq[rk]) if ghi < N - 1 else np.full(_QB, np.inf)
            guard = np.minimum(lo_m, hi_m)
            bad = ~((guard >= 0) & (nn[j * _QB + arange_qb] <= guard * guard))
            uncert[j * _QB + arange_qb] = bad

        nu = int(uncert.sum())
        if nu:
            # exact host recompute against the batch's full ref set
            qu = qs[:, h * _NQ_CORE + np.nonzero(uncert)[0]]
            nn[uncert] = _exact_nn(qu, rs).astype(np.float64)
        total += nn.sum()

    loss = (2.0 / _B) * total
    return np.float32(loss)
